# revision 1
# baseline (speedup 1.0000x reference)
"""Trainium2 Bass kernel for nn_Local_EncoderLayer (local+global sparse attention encoder).

Sharding: data-parallel over batch B=8 across 8 cores (one batch per core).
Both attention stages and the local/global regroup are batch-internal, so
there is no cross-core communication.

v2 design vs v0:
- Activations resident as X^T [D, T] in bf16 (32KB/partition); weights cast
  to bf16 on host and DMA'd ONCE per stage into SBUF (attn: 64KB, FFN:
  128KB per partition) instead of re-streamed per pass (4x less HBM traffic).
- Attention fused per 512-token tile: QK production -> per-128-group
  V/softmax/attn@V -> proj -> LN, pipelined across tiles so PE-heavy QK/proj
  overlaps DVE/ACT-heavy softmax of neighbouring tiles.
- FFN restructured to 256-token tiles with the d_inner loop innermost and the
  output accumulated in PSUM (start/stop flags) - eliminates the SBUF Yacc
  accumulator and all its DVE adds.
- Block-diagonal mask folded into the score matmul as extra contraction rows
  (rank trick, additive -50 off-block). Softmax uses ACT exp with accum_out;
  P is transposed on the PE and "attn@V - V" computed as V^T(P^T - I).
- LayerNorm cross-partition mean/var via ones-vector matmuls + broadcast
  matmul, all moving operands >= 256 wide (full fp32r/bf16 PE rate).
"""
import functools
import numpy as np
import ml_dtypes

import concourse.bass as bass
import concourse.tile as tile
from concourse import bacc, mybir
from concourse.bass import ds
from concourse.bass_utils import run_bass_kernel_spmd

B, L, D, H, DK, DV, DI, NL = 8, 2048, 1024, 16, 64, 64, 4096, 32
EPS = 1e-5
P = 128
T = L                   # tokens per core
DC = D // P             # 8 d-chunks
FC = (H * DK) // P      # 8 head-pair chunks
NT = T // 512           # 4 attention tiles of 512 tokens
NG = 512 // P           # 4 groups (of 128 tokens) per attention tile
GSEQ = T // NL          # global-stage sequence length (64)
SPG = P // GSEQ         # sequences per group in global stage (2)
FW = 256                # ffn tile width (tokens)
NFT = T // FW           # 8 ffn tiles
NDIC = DI // 512        # 8 ffn inner chunks
LW = 256                # layernorm chunk width

F32 = mybir.dt.float32
F32R = mybir.dt.float32r
BF16 = mybir.dt.bfloat16
AF = mybir.ActivationFunctionType
ALU = mybir.AluOpType
MASK_C = float(np.sqrt(50.0))

PHASE_MARKS = []  # (phase_name, instruction_count_at_entry) — for offline profiling


def _mark(nc, name):
    PHASE_MARKS.append((name, nc.get_next_instruction_name()))


def _din(nc, name, shape, dt=F32):
    return nc.dram_tensor(name, shape, dt, kind="ExternalInput").ap()


def _build_nc(repeat=1):
    nc = bacc.Bacc("TRN2", target_bir_lowering=False, debug=False, num_devices=8)
    # x / y are supplied and returned TRANSPOSED ([D, T]) — the host does the
    # [T, D] <-> [D, T] transposes so the device skips both transpose passes.
    x_in = _din(nc, "x", [D, T], BF16)
    y_out = nc.dram_tensor("y", [D, T], BF16, kind="ExternalOutput").ap()

    W = {}
    for pfx in ("la", "sa"):
        W[pfx] = dict(
            wq=_din(nc, f"{pfx}_wq", [D, H * DK], BF16),
            wk=_din(nc, f"{pfx}_wk", [D, H * DK], BF16),
            wv=_din(nc, f"{pfx}_wv", [D, H * DV], BF16),
            pw=_din(nc, f"{pfx}_pw", [H * DV, D], BF16),
            pb=_din(nc, f"{pfx}_pb", [D]),
            g=_din(nc, f"{pfx}_g", [D]),
            b=_din(nc, f"{pfx}_b", [D]),
        )
    for pfx in ("lf", "pf"):
        W[pfx] = dict(
            w1=_din(nc, f"{pfx}_w1", [D, DI], BF16),
            b1=_din(nc, f"{pfx}_b1", [DI]),
            w2=_din(nc, f"{pfx}_w2", [DI, D], BF16),
            b2=_din(nc, f"{pfx}_b2", [D]),
            g=_din(nc, f"{pfx}_g", [D]),
            b=_din(nc, f"{pfx}_b", [D]),
        )
    idbf = _din(nc, "idbf", [P, P], BF16)
    mq_l = _din(nc, "mq_l", [NL // 8 + 1, P], BF16)   # 5 rows
    mk_l = _din(nc, "mk_l", [NL // 8 + 1, P], BF16)
    mq_g = _din(nc, "mq_g", [SPG + 1, P], BF16)
    mk_g = _din(nc, "mk_g", [SPG + 1, P], BF16)
    ones_col = _din(nc, "ones_col", [P, 1], BF16)
    eps_col = _din(nc, "eps_col", [P, 1], F32)
    invd_row = _din(nc, "invd_row", [1, P], F32R)

    with tile.TileContext(nc) as tc:
        for _rep in range(repeat):
            _body(nc, tc, x_in, y_out, W,
                  dict(idbf=idbf, mq_l=mq_l, mk_l=mk_l, mq_g=mq_g, mk_g=mk_g,
                       ones_col=ones_col, invd_row=invd_row, eps_col=eps_col))
    nc.compile()
    return nc


def _body(nc, tc, x_in, y_out, W, consts):
    from contextlib import ExitStack
    ctx = ExitStack()
    with ctx:
        cp = ctx.enter_context(tc.tile_pool(name="const", bufs=1))
        xp = ctx.enter_context(tc.tile_pool(name="xres", bufs=1))

        # ---- consts to SBUF
        def cload(name, shape, dt):
            t = cp.tile(shape, dt, tag=name, name=name)
            nc.sync.dma_start(t[:], consts[name])
            return t
        idbf_t = cload("idbf", [P, P], BF16)
        mq_l_t = cload("mq_l", [5, P], BF16)
        mk_l_t = cload("mk_l", [5, P], BF16)
        mq_g_t = cload("mq_g", [SPG + 1, P], BF16)
        mk_g_t = cload("mk_g", [SPG + 1, P], BF16)
        ones_t = cload("ones_col", [P, 1], BF16)
        invd_t = cload("invd_row", [1, P], F32R)
        eps_t = cload("eps_col", [P, 1], F32)

        def vec_tile(ap, n, name):
            # [n] dram vector -> [P, n//P] sbuf tile (col c = chunk c)
            t = cp.tile([P, n // P], F32, tag=name, name=name)
            nc.sync.dma_start(t[:], ap.rearrange("(c p) -> p c", p=P))
            return t
        VT = {}
        for pfx in ("la", "sa"):
            VT[pfx] = dict(
                pb=vec_tile(W[pfx]["pb"], D, f"{pfx}_pb"),
                g=vec_tile(W[pfx]["g"], D, f"{pfx}_g"),
                b=vec_tile(W[pfx]["b"], D, f"{pfx}_b"),
            )
        for pfx in ("lf", "pf"):
            VT[pfx] = dict(
                b1=vec_tile(W[pfx]["b1"], DI, f"{pfx}_b1"),
                b2=vec_tile(W[pfx]["b2"], D, f"{pfx}_b2"),
                g=vec_tile(W[pfx]["g"], D, f"{pfx}_g"),
                b=vec_tile(W[pfx]["b"], D, f"{pfx}_b"),
            )

        # ---- resident X^T tiles (bf16)
        X = [xp.tile([P, T], BF16, tag=f"x{dc}", name=f"x{dc}") for dc in range(DC)]

        def xcols(dc, glob, j0, n):
            """AP view of X[dc] columns for (grouped) token range [j0, j0+n)."""
            if not glob:
                return X[dc][:, ds(j0, n)]
            # grouped index j = GSEQ*s + k ; token t = s + NL*k
            Xr = X[dc].rearrange("p (k s) -> p s k", s=NL)  # [P, 32, 64]
            return Xr[:, j0 // GSEQ: (j0 + n) // GSEQ, :]

        def gv(ap, glob):
            """Reshape a contiguous [P, n] view to [P, n//GSEQ, GSEQ] to match strided views."""
            if not glob:
                return ap
            return ap.rearrange("p (a b) -> p a b", b=GSEQ)

        _mark(nc, "in")
        # ---- input: x^T [D, T] bf16 straight into the resident tiles
        for dc in range(DC):
            nc.sync.dma_start(X[dc][:], x_in[ds(P * dc, P), :])

        # ---- layernorm over X columns [j0, j0+w), in LW chunks.
        # Single psum bank, fully sequential: each accumulation group starts
        # only after the previous group's psum readers have drained.
        def layer_norm(g_t, b_t, glob, j0, w, lnp, lnps):
            for c0 in range(j0, j0 + w, LW):
                lpk = lnps.tile([P, 512], F32, tag="lpk", bufs=1)
                for dc in range(DC):
                    zsl = xcols(dc, glob, c0, LW)
                    nc.tensor.matmul(lpk[0:1, 0:LW], ones_t[:], zsl,
                                     start=(dc == 0), stop=(dc == DC - 1))
                s1s = lnp.tile([1, LW], F32R, tag="s1s", bufs=1)
                nc.vector.tensor_copy(s1s[:], lpk[0:1, 0:LW])
                nc.tensor.matmul(lpk[:, ds(LW, LW)], invd_t[:], s1s[:],
                                 start=True, stop=True)
                mu_sb = lnp.tile([P, LW], F32, tag="mu_sb", bufs=1)
                nc.vector.tensor_copy(mu_sb[:], lpk[:, ds(LW, LW)])
                for dc in range(DC):
                    zsl = xcols(dc, glob, c0, LW)
                    sq_t = lnp.tile([P, LW], BF16, tag="sqt", bufs=2)
                    nc.vector.tensor_mul(gv(sq_t[:], glob), zsl, zsl)
                    nc.tensor.matmul(lpk[0:1, 0:LW], ones_t[:], sq_t[:],
                                     start=(dc == 0), stop=(dc == DC - 1))
                sqs = lnp.tile([1, LW], F32R, tag="sqs", bufs=1)
                nc.vector.tensor_copy(sqs[:], lpk[0:1, 0:LW])
                nc.tensor.matmul(lpk[:, ds(LW, LW)], invd_t[:], sqs[:],
                                 start=True, stop=True)
                sqb = lpk[:, ds(LW, LW)]
                mu2 = lnp.tile([P, LW], F32, tag="mu2", bufs=1)
                nc.vector.tensor_mul(mu2[:], mu_sb[:], mu_sb[:])
                var = lnp.tile([P, LW], F32, tag="var", bufs=1)
                nc.vector.scalar_tensor_tensor(var[:], mu2[:], -1.0, sqb,
                                               op0=ALU.mult, op1=ALU.add)
                sig = lnp.tile([P, LW], F32, tag="sig", bufs=1)
                nc.scalar.activation(sig[:], var[:], AF.Sqrt, bias=eps_t[:])
                rsig = lnp.tile([P, LW], F32, tag="rsig", bufs=1)
                nc.vector.reciprocal(rsig[:], sig[:])
                for dc in range(DC):
                    zsl = xcols(dc, glob, c0, LW)
                    nc.vector.scalar_tensor_tensor(zsl, zsl, 0.0, gv(mu_sb[:], glob),
                                                   op0=ALU.add, op1=ALU.subtract)
                    nc.vector.scalar_tensor_tensor(zsl, zsl, g_t[:, dc:dc + 1],
                                                   gv(rsig[:], glob),
                                                   op0=ALU.mult, op1=ALU.mult)
                    nc.vector.tensor_scalar_add(zsl, zsl, b_t[:, dc:dc + 1])

        # ---- attention stage (weights resident, fused per 512-token tile)
        def attn_stage(pfx, glob):
            w = W[pfx]
            vt = VT[pfx]
            mq_t, mk_t = (mq_g_t, mk_g_t) if glob else (mq_l_t, mk_l_t)
            from contextlib import ExitStack
            sctx = ExitStack()
            with sctx:
                wpool = sctx.enter_context(tc.tile_pool(name=f"{pfx}_w", bufs=1))
                bp = sctx.enter_context(tc.tile_pool(name=f"{pfx}_buf", bufs=1))
                sp = sctx.enter_context(tc.tile_pool(name=f"{pfx}_small", bufs=1))
                lnp = sctx.enter_context(tc.tile_pool(name=f"{pfx}_ln", bufs=1))
                qkps = sctx.enter_context(
                    tc.tile_pool(name=f"{pfx}_qkps", bufs=1, space="PSUM"))
                aps = sctx.enter_context(
                    tc.tile_pool(name=f"{pfx}_aps", bufs=1, space="PSUM"))
                zpsp = sctx.enter_context(
                    tc.tile_pool(name=f"{pfx}_zps", bufs=1, space="PSUM"))
                lnps = sctx.enter_context(
                    tc.tile_pool(name=f"{pfx}_lnps", bufs=1, space="PSUM"))

                # resident weights (loaded once)
                wqc = [wpool.tile([P, H * DK], BF16, tag=f"wq{dc}", name=f"wq{dc}")
                       for dc in range(DC)]
                wkc = [wpool.tile([P, H * DK], BF16, tag=f"wk{dc}", name=f"wk{dc}")
                       for dc in range(DC)]
                wvc = [wpool.tile([P, H * DV], BF16, tag=f"wv{dc}", name=f"wv{dc}")
                       for dc in range(DC)]
                pwc = [wpool.tile([P, D], BF16, tag=f"pw{kc}", name=f"pw{kc}")
                       for kc in range(FC)]
                # load order matters: the first QK chains need all of wq/wk
                # before wv/pw are touched
                for dc in range(DC):
                    nc.sync.dma_start(wqc[dc][:], w["wq"][ds(P * dc, P), :])
                for dc in range(DC):
                    nc.sync.dma_start(wkc[dc][:], w["wk"][ds(P * dc, P), :])
                for dc in range(DC):
                    nc.sync.dma_start(wvc[dc][:], w["wv"][ds(P * dc, P), :])
                for dc in range(DC):
                    nc.sync.dma_start(pwc[dc][:], w["pw"][ds(P * dc, P), :])

                # ---- group-level software pipeline over all 16 groups.
                # Engines execute their queues IN ORDER, so each iteration
                # emits, per head-slot: phase-1 ops of group gg (scores/exp/
                # softmax-normalize) interleaved with phase-2 ops of group
                # gg-1 (transpose/attn@V) whose inputs are already ready,
                # then the proj of group gg-1 as a block of ready matmuls.
                NGT = NG * NT
                QTKT = {}
                attTs = {}
                prev = None

                def emit_qk(t):
                    j0 = 512 * t
                    QT = [bp.tile([P, 512], BF16, tag=f"qt{fc}", name=f"qt{fc}", bufs=2)
                          for fc in range(FC)]
                    KT = [bp.tile([P, 512], BF16, tag=f"kt{fc}", name=f"kt{fc}", bufs=2)
                          for fc in range(FC)]
                    attT = [bp.tile([P, 512], BF16, tag=f"at{kc}", name=f"at{kc}", bufs=2)
                            for kc in range(FC)]
                    for wch, dstl in ((wqc, QT), (wkc, KT)):
                        for fc in range(FC):
                            ps = qkps.tile([P, 512], F32, tag="qk", bufs=2)
                            for dc in range(DC):
                                nc.tensor.matmul(
                                    ps[:], wch[dc][:, ds(P * fc, P)],
                                    xcols(dc, glob, j0, 512),
                                    start=(dc == 0), stop=(dc == DC - 1))
                            nc.scalar.activation(dstl[fc][:], ps[:], AF.Copy)
                    QTKT[t] = (QT, KT)
                    attTs[t] = attT

                for gg in range(NGT + 2):
                    t, g = divmod(gg, NG)
                    live = gg < NGT
                    if live and g == 0:
                        emit_qk(t)
                    if live:
                        gj = P * gg
                        QT, KT = QTKT[t]
                        attT = attTs[t]
                        if glob:
                            xg_st = sp.tile([P, DC, P], BF16, tag="xgst", bufs=2)
                            for dc in range(DC):
                                if dc % 2:
                                    nc.scalar.activation(
                                        gv(xg_st[:, dc, :], glob),
                                        xcols(dc, glob, gj, P), AF.Copy)
                                else:
                                    nc.vector.tensor_copy(
                                        gv(xg_st[:, dc, :], glob),
                                        xcols(dc, glob, gj, P))
                        v_t = sp.tile([P, H * DV], BF16, tag="v", bufs=2)
                        for hf in range(2):
                            vps = qkps.tile([P, 512], F32, tag="qk", bufs=2)
                            for dc in range(DC):
                                xg = xg_st[:, dc, :] if glob else xcols(dc, glob, gj, P)
                                nc.tensor.matmul(
                                    vps[:], xg, wvc[dc][:, ds(512 * hf, 512)],
                                    start=(dc == 0), stop=(dc == DC - 1))
                            nc.scalar.activation(
                                v_t[:, ds(512 * hf, 512)], vps[:], AF.Copy)
                        pn_set = sp.tile([P, H, P], BF16, tag="pn", bufs=2)

                    ptns = {}
                    for h in range(H + 2):
                        # phase-2 for the previous group, itself pipelined:
                        # transpose(h) feeds DVE, attn@V consumes ptn(h-2)
                        # which is ready — the PE never waits inside a slot.
                        if prev is not None:
                            p_gg, p_vt, p_pn, p_t, p_g = prev
                            p_attT = attTs[p_t]
                            if h < H:
                                pt_ps = aps.tile([P, P], BF16, tag="pt",
                                                 bufs=1, name="pt_ps")
                                nc.tensor.transpose(pt_ps[:], p_pn[:, h, :],
                                                    idbf_t[:])
                                ptn = sp.tile([P, P], BF16, tag="ptn", bufs=4)
                                nc.vector.scalar_tensor_tensor(
                                    ptn[:], pt_ps[:], 1.0, idbf_t[:],
                                    op0=ALU.mult, op1=ALU.subtract)
                                ptns[h] = ptn
                            if h >= 2:
                                hv = h - 2
                                fc, hi = divmod(hv, 2)
                                o_tile = aps.tile([64, P], F32, tag="o",
                                                  bufs=1, name="o_tile")
                                o_ps = o_tile[:]
                                nc.tensor.matmul(o_ps, p_vt[:, ds(64 * hv, 64)],
                                                 ptns.pop(hv)[:],
                                                 start=True, stop=True)
                                dst = p_attT[fc][64 * hi:64 * hi + 64,
                                                 ds(P * p_g, P)]
                                if hv % 2:
                                    nc.scalar.activation(dst, o_ps, AF.Copy)
                                else:
                                    nc.vector.tensor_copy(dst, o_ps)
                        # proj + residual of group gg-2, one 8-matmul
                        # chain every other slot — the interleaved slot ops
                        # give the residual DVE read time to drain the bank
                        if h % 2 == 0 and h < H and 0 <= gg - 2 < NGT:
                            oc = h // 2
                            pp_gg = gg - 2
                            pp_t, pp_g = divmod(pp_gg, NG)
                            pp_attT = attTs[pp_t]
                            zsl = zpsp.tile([P, P], F32, tag="z", bufs=1)
                            for kc in range(FC):
                                nc.tensor.matmul(
                                    zsl[:], pwc[kc][:, ds(P * oc, P)],
                                    pp_attT[kc][:, ds(P * pp_g, P)],
                                    start=(kc == 0), stop=(kc == FC - 1))
                            xd = xcols(oc, glob, P * pp_gg, P)
                            nc.vector.scalar_tensor_tensor(
                                xd, gv(zsl[:], glob),
                                vt["pb"][:, oc:oc + 1], xd,
                                op0=ALU.add, op1=ALU.add)
                        # phase-1 ops for the current group
                        if live and h < H:
                            fc, hi = divmod(h, 2)
                            s_tile = aps.tile([P, P], F32, tag="s", bufs=2,
                                              name="s_tile")
                            s_ps = s_tile[:]
                            nc.tensor.matmul(
                                s_ps,
                                QT[fc][64 * hi:64 * hi + 64, ds(P * g, P)],
                                KT[fc][64 * hi:64 * hi + 64, ds(P * g, P)],
                                start=True, stop=False)
                            nc.tensor.matmul(s_ps, mq_t[:], mk_t[:],
                                             start=False, stop=True)
                            pexp = sp.tile([P, P], BF16, tag="pexp", bufs=4)
                            ssum = sp.tile([P, 1], F32, tag="ssum", bufs=8)
                            nc.scalar.activation(pexp[:], s_ps, AF.Exp,
                                                 accum_out=ssum[:])
                            srec = sp.tile([P, 1], F32, tag="srec", bufs=8)
                            nc.vector.reciprocal(srec[:], ssum[:])
                            nc.vector.tensor_scalar_mul(
                                pn_set[:, h, :], pexp[:], srec[:])

                    # --- tile t-1's LN, two groups after its last residual
                    if gg % NG == 1 and gg > NG:
                        layer_norm(vt["g"], vt["b"], glob, 512 * (t - 1), 512,
                                   lnp, lnps)
                    prev = (gg, v_t, pn_set, t, g) if live else None
                layer_norm(vt["g"], vt["b"], glob, 512 * (NT - 1), 512, lnp, lnps)
        # ---- FFN stage (w1/w2 resident, PSUM-accumulated output)
        def ffn_stage(pfx):
            w = W[pfx]
            vt = VT[pfx]
            from contextlib import ExitStack
            sctx = ExitStack()
            with sctx:
                wpool = sctx.enter_context(tc.tile_pool(name=f"{pfx}_w", bufs=1))
                hp = sctx.enter_context(tc.tile_pool(name=f"{pfx}_h", bufs=1))
                lnp = sctx.enter_context(tc.tile_pool(name=f"{pfx}_ln", bufs=1))
                hps_p = sctx.enter_context(
                    tc.tile_pool(name=f"{pfx}_hps", bufs=1, space="PSUM"))
                yps_p = sctx.enter_context(
                    tc.tile_pool(name=f"{pfx}_yps", bufs=1, space="PSUM"))
                lnps = sctx.enter_context(
                    tc.tile_pool(name=f"{pfx}_lnps", bufs=1, space="PSUM"))

                w1c = [wpool.tile([P, DI], BF16, tag=f"w1{dc}", name=f"w1{dc}")
                       for dc in range(DC)]
                w2c = [wpool.tile([P, D], BF16, tag=f"w2{kc}", name=f"w2{kc}")
                       for kc in range(DI // P)]
                for dc in range(DC):
                    nc.sync.dma_start(w1c[dc][:], w["w1"][ds(P * dc, P), :])
                for kc in range(DI // P):
                    nc.sync.dma_start(w2c[kc][:], w["w2"][ds(P * kc, P), :])

                pend_ln = None
                for t in range(NFT):
                    j0 = FW * t
                    # --- produce all of H = relu(x@w1 + b1) for this tile
                    hsb = hp.tile([P, DI // P, FW], BF16, tag="hsb", bufs=1)
                    for d in range(NDIC):
                        # separate per-chain psum allocations (bank each,
                        # 4-deep rotation) so chain r+1 never serializes
                        # behind the relu read of chain r
                        for r in range(4):
                            hps = hps_p.tile([P, 512], F32, tag="h", bufs=4)
                            for dc in range(DC):
                                nc.tensor.matmul(
                                    hps[0:P, 0:FW],
                                    w1c[dc][:, ds(512 * d + P * r, P)],
                                    X[dc][:, ds(j0, FW)],
                                    start=(dc == 0), stop=(dc == DC - 1))
                            nc.scalar.activation(
                                hsb[:, 4 * d + r, :], hps[0:P, 0:FW], AF.Relu,
                                bias=vt["b1"][:, 4 * d + r:4 * d + r + 1])
                    # --- deferred LN of previous tile overlaps H production
                    if pend_ln is not None:
                        layer_norm(vt["g"], vt["b"], False, pend_ln, FW, lnp, lnps)
                    # --- y = h@w2, one psum chain per oc, rotating 2 banks
                    for oc in range(DC):
                        ysl = yps_p.tile([P, FW], F32, tag="y", bufs=2)
                        for kc in range(DI // P):
                            nc.tensor.matmul(
                                ysl[:], w2c[kc][:, ds(P * oc, P)], hsb[:, kc, :],
                                start=(kc == 0), stop=(kc == DI // P - 1))
                        # residual + b2 -> X
                        xd = X[oc][:, ds(j0, FW)]
                        nc.vector.scalar_tensor_tensor(
                            xd, ysl[:], vt["b2"][:, oc:oc + 1], xd,
                            op0=ALU.add, op1=ALU.add)
                    pend_ln = j0
                layer_norm(vt["g"], vt["b"], False, pend_ln, FW, lnp, lnps)

        import os
        parts = os.environ.get("KPARTS", "full")
        _mark(nc, "la")
        if parts in ("full", "la", "la_lf"):
            attn_stage("la", glob=False)
        _mark(nc, "lf")
        if parts in ("full", "lf", "la_lf"):
            ffn_stage("lf")
        _mark(nc, "sa")
        if parts == "full":
            attn_stage("sa", glob=True)
        _mark(nc, "pf")
        if parts == "full":
            ffn_stage("pf")
        _mark(nc, "out")

        # ---- output: X^T -> y^T [D, T] bf16 (host re-transposes)
        for dc in range(DC):
            nc.sync.dma_start(y_out[ds(P * dc, P), :], X[dc][:])


# ------------------------------------------------------------------ host side

def _host_consts():
    r = MASK_C
    nloc = NL // 8 + 1  # 5
    mq_l = np.zeros((nloc, P), np.float32)
    mk_l = np.zeros((nloc, P), np.float32)
    for blk in range(P // NL):
        mq_l[blk, blk * NL:(blk + 1) * NL] = r
        mk_l[blk, blk * NL:(blk + 1) * NL] = r
    mq_l[-1, :] = r
    mk_l[-1, :] = -r
    mq_g = np.zeros((SPG + 1, P), np.float32)
    mk_g = np.zeros((SPG + 1, P), np.float32)
    for blk in range(SPG):
        mq_g[blk, blk * GSEQ:(blk + 1) * GSEQ] = r
        mk_g[blk, blk * GSEQ:(blk + 1) * GSEQ] = r
    mq_g[-1, :] = r
    mk_g[-1, :] = -r
    bf = ml_dtypes.bfloat16
    return dict(
        idbf=np.eye(P, dtype=bf),
        mq_l=mq_l.astype(bf), mk_l=mk_l.astype(bf),
        mq_g=mq_g.astype(bf), mk_g=mk_g.astype(bf),
        ones_col=np.ones((P, 1), bf),
        eps_col=np.full((P, 1), EPS, np.float32),
        invd_row=np.full((1, P), 1.0 / D, np.float32),
    )


@functools.lru_cache(maxsize=2)
def _get_nc(repeat=1):
    return _build_nc(repeat)


def _shared_inputs(inputs):
    bf = ml_dtypes.bfloat16
    sh = {}
    for pfx in ("la", "sa"):
        sh[f"{pfx}_wq"] = np.ascontiguousarray(
            inputs[f"{pfx}_wqs"].transpose(1, 0, 2).reshape(D, H * DK)
            * 0.125).astype(bf)
        sh[f"{pfx}_wk"] = np.ascontiguousarray(
            inputs[f"{pfx}_wks"].transpose(1, 0, 2).reshape(D, H * DK)).astype(bf)
        sh[f"{pfx}_wv"] = np.ascontiguousarray(
            inputs[f"{pfx}_wvs"].transpose(1, 0, 2).reshape(D, H * DV)).astype(bf)
        sh[f"{pfx}_pw"] = np.ascontiguousarray(inputs[f"{pfx}_pw"]).astype(bf)
        sh[f"{pfx}_pb"] = np.ascontiguousarray(inputs[f"{pfx}_pb"], np.float32)
        sh[f"{pfx}_g"] = np.ascontiguousarray(inputs[f"{pfx}_g"], np.float32)
        sh[f"{pfx}_b"] = np.ascontiguousarray(inputs[f"{pfx}_b"], np.float32)
    for pfx in ("lf", "pf"):
        sh[f"{pfx}_w1"] = np.ascontiguousarray(inputs[f"{pfx}_w1"]).astype(bf)
        sh[f"{pfx}_w2"] = np.ascontiguousarray(inputs[f"{pfx}_w2"]).astype(bf)
        for k in ("b1", "b2", "g", "b"):
            sh[f"{pfx}_{k}"] = np.ascontiguousarray(inputs[f"{pfx}_{k}"], np.float32)
    sh.update(_host_consts())
    return sh


def kernel(**inputs):
    nc = _get_nc()
    sh = _shared_inputs(inputs)
    x = np.asarray(inputs["enc_input"]).astype(ml_dtypes.bfloat16)
    in_maps = []
    for c in range(B):
        m = dict(sh)
        m["x"] = np.ascontiguousarray(x[c].T)
        in_maps.append(m)
    res = run_bass_kernel_spmd(nc, in_maps, core_ids=list(range(B)))
    return np.stack([res.results[c]["y"].T for c in range(B)], axis=0).astype(np.float32)

